# revision 24
# baseline (speedup 1.0000x reference)
"""AttentionConv3D Trainium2 kernel.

Computation (per channel c, voxel (d,h,w)):
    q,k,v = 1x1x1 convs of x;  s_kv = q * (k_pad[nbr kv] + rel_bias(c,kv))
    out   = sum_kv softmax_kv(s) * v_pad[nbr kv]         (27 = 3x3x3 window)

Strategy: depth-shard over 8 cores (2 output depth planes each, 1-plane halo).
On-device layout: partition p = dl*64 + c (dl in {0,1} local depth), free dim
= zero-padded 66x66 plane (4356). Per kv-neighbor the window access is a
free-dim offset (kh-1)*66 + (kw-1) into one of three depth-plane buffers
K[kd]; the rel bias collapses to a per-partition scalar B[p, kv], so
s = (K_shift + B)*q is ONE DVE scalar_tensor_tensor op. exp on ACT;
numerator/denominator accumulated with identity matmuls into PSUM on the
TensorEngine; 1/den via exp(-ln(den)) on ACT.

The wall clock is dominated by the ~40MB/s (half-duplex) axon tunnel, so I/O
is minimized:
 - each core uploads ONLY its two fp16 depth planes (no halo duplication);
   the 1-plane halos are exchanged on device: a world AllGather of every
   core's plane pair, then a per-core one-hot masked sum (host-uploaded
   selection scalars, 16 DVE select-accumulate ops) picks the two neighbor
   planes — edge cores get all-zero masks, i.e. free zero padding.
 - projection matmuls run fp16 x fp16 -> fp32 PSUM.
 - the output is downloaded fp16 and upconverted on host.
 - donated output buffers are device-resident (recycled between calls).
 - the jitted dispatch closure is built once and cached.
"""

import sys
import numpy as np

for _p in ("/opt/trn_rl_repo", "/root/.axon_site/_ro/trn_rl_repo"):
    if _p not in sys.path:
        sys.path.insert(0, _p)

HP = 66               # padded plane edge
HW = HP * HP          # 4356
CHUNKS = [(67, 1386), (1453, 1452), (2905, 1384)]  # covers [67, 4289), chunk
# edges row-aligned so each output row band is quantized by exactly one
# chunk's int8 scale. Chunk 0's window reads ([0, 1520)) still fit inside proj
# col-chunk 0 so the kv loop overlaps the tail of the projection phase.
PROJ = [(0, 1536), (1536, 1536), (3072, 1284)]     # proj psum chunks over 4356
OUT_ROWS = [(0, 21), (21, 43), (43, 64)]           # row bands DMA'd per chunk
ZW = 512              # halo-assembly column chunk (8 chunks cover 4096)

# hot-path dtype knobs (fp32 = safe; bf16 halves DVE cost of the e*v path)
E_BF16 = True   # e / v / ev tiles + identity in bf16 (PE still accums fp32)

N_CORES = 8

_CACHE = {}


def _subs(L):
    return [(0, 512), (512, 512), (1024, L - 1024)]


def _build():
    from contextlib import ExitStack
    import concourse.bacc as bacc
    import concourse.tile as tile
    from concourse import mybir

    f32 = mybir.dt.float32
    f16 = mybir.dt.float16
    bf16 = mybir.dt.bfloat16
    edt = bf16 if E_BF16 else f32
    Alu = mybir.AluOpType
    Act = mybir.ActivationFunctionType

    nc = bacc.Bacc("TRN2", target_bir_lowering=False)
    # own two unpadded depth planes, partition = slot*64 + channel
    bnd_d = nc.dram_tensor("bnd", [128, 4096], f16, kind="ExternalInput")
    # halo selection scalars: rows 0:64 pick the left-halo plane, 64:128 the
    # right-halo plane, as one-hot over the 16 gathered planes
    hs_d = nc.dram_tensor("hsel", [128, 16], f32, kind="ExternalInput")
    wk_d = nc.dram_tensor("wk2", [64, 128], f16, kind="ExternalInput")
    wv_d = nc.dram_tensor("wv2", [64, 128], f16, kind="ExternalInput")
    wq_d = nc.dram_tensor("wq2", [64, 128], f16, kind="ExternalInput")
    b_d = nc.dram_tensor("bias", [128, 27], f32, kind="ExternalInput")
    id_d = nc.dram_tensor("ident", [128, 128], edt, kind="ExternalInput")
    i8 = mybir.dt.int8
    # every core receives the full gathered result; the host fetches only
    # core 0's copy (one transfer instead of 16 per-shard fetches)
    out_d = nc.dram_tensor("out", [N_CORES * 128, 4112], i8,
                           kind="ExternalOutput")

    # collective staging (collectives can't touch I/O tensors directly)
    bb_d = nc.dram_tensor("bb", [128, 4096], f16)
    g_d = nc.dram_tensor("g", [16, 64, 4096], f16, addr_space="Shared")
    # per-core packed result: 64*64 int8 voxels + 16 bytes of f32 scales
    oc_d = nc.dram_tensor("oc", [128, 4112], i8)
    og_d = nc.dram_tensor("og", [N_CORES * 128, 4112], i8, addr_space="Shared")

    with tile.TileContext(nc) as tc, ExitStack() as ctx:
        singles = ctx.enter_context(tc.tile_pool(name="singles", bufs=1))
        planes = ctx.enter_context(tc.tile_pool(name="planes", bufs=1))
        wpool = ctx.enter_context(tc.tile_pool(name="work", bufs=2))

        wk_s = singles.tile([64, 128], f16, tag="wk")
        wv_s = singles.tile([64, 128], f16, tag="wv")
        wq_s = singles.tile([64, 128], f16, tag="wq")
        id_s = singles.tile([128, 128], edt, tag="id")
        b_s = singles.tile([128, 27], f32, tag="b")
        hs_s = singles.tile([128, 16], f32, tag="hs")
        ebias = singles.tile([128, 1], f32, tag="ebias")
        nc.vector.memset(ebias[:], -28.0)
        for t, d in ((wk_s, wk_d), (wv_s, wv_d), (wq_s, wq_d),
                     (id_s, id_d), (b_s, b_d), (hs_s, hs_d)):
            nc.sync.dma_start(t[:], d[:])

        # ---- halo exchange: world AllGather of everyone's plane pair
        nc.gpsimd.dma_start(bb_d[:], bnd_d[:])
        nc.gpsimd.collective_compute(
            "AllGather", mybir.AluOpType.bypass,
            replica_groups=[list(range(N_CORES))],
            ins=[bb_d[:]], outs=[g_d[:]])

        # the four padded depth planes, assembled from unpadded data: memset
        # zeroes the 66x66 borders, the interiors land via strided DMAs
        XP = [planes.tile([64, HW], f16, tag=f"xp{m}", name=f"xp{m}")
              for m in range(4)]
        XPv = [t.rearrange("p (r c) -> p r c", c=HP) for t in XP]
        for t in XP:
            nc.vector.memset(t[:], 0.0)
        bnd_v = bnd_d[:].rearrange("p (r c) -> p r c", c=64)
        nc.sync.dma_start(XPv[1][:, 1:65, 1:65], bnd_v[0:64])
        nc.sync.dma_start(XPv[2][:, 1:65, 1:65], bnd_v[64:128])

        with tc.tile_pool(name="gt", bufs=1) as gpool:
            # 0:64 = left halo plane, 64:128 = right halo plane (unpadded)
            XH = gpool.tile([128, 4096], f16, tag="xh")
            for w in range(8):
                ws = w * ZW
                GT = gpool.tile([128, 16 * ZW], f16, tag="gt")
                GTv = GT.rearrange("p (j z) -> p j z", j=16)
                src = g_d[:, :, ws:ws + ZW].transpose([1, 0, 2])
                nc.sync.dma_start(GTv[0:64, :, :], src)
                nc.sync.dma_start(GTv[64:128, :, :], src)
                # one-hot select-accumulate over the 16 gathered planes
                nc.vector.tensor_scalar_mul(
                    XH[:, ws:ws + ZW], GT[:, 0:ZW], hs_s[:, 0:1])
                for j in range(1, 16):
                    nc.vector.scalar_tensor_tensor(
                        XH[:, ws:ws + ZW], GT[:, j * ZW:(j + 1) * ZW],
                        hs_s[:, j:j + 1], XH[:, ws:ws + ZW],
                        Alu.mult, Alu.add)
            # pad the halo planes (and bring the right halo down to base
            # partition 0, which matmul moving operands require)
            XHv = XH.rearrange("p (r c) -> p r c", c=64)
            nc.sync.dma_start(XPv[0][:, 1:65, 1:65], XHv[0:64])
            nc.sync.dma_start(XPv[3][:, 1:65, 1:65], XHv[64:128])

        Kp = [planes.tile([128, HW], f32, tag=f"k{i}", name=f"k{i}") for i in range(3)]
        Vp = [planes.tile([128, HW], edt, tag=f"v{i}", name=f"v{i}") for i in range(3)]
        Q = planes.tile([128, HW], f32, tag="q")
        OUT = planes.tile([128, HW], i8, tag="o")
        # int8 quantization state: per-chunk per-partition scale = 126/absmax
        am_s = singles.tile([128, 4], f32, tag="am")
        rc_s = singles.tile([128, 4], f32, tag="rc")
        nc.vector.memset(rc_s[:], 1.0)

        # ---- projections: plane m -> k/v (dual-copy weights give the same
        # output plane on partitions 0:64 and 64:128), q for m in {1,2}.
        # column-chunk OUTER so all planes' first 1536 columns (what kv chunk 0
        # needs) are projected before any plane's later columns.
        Xsrc = [t[:] for t in XP]
        with tc.tile_pool(name="pp", bufs=2, space="PSUM") as ppool:
            for base, L3 in PROJ:
                for m in range(4):
                    X = Xsrc[m]
                    projs = [(wk_s, "k"), (wv_s, "v")]
                    if m in (1, 2):
                        projs.append((wq_s, "q"))
                    for w_s, kind in projs:
                        pp = ppool.tile([128, 1536], f32, tag="pp")
                        for a, bl in _subs(L3):
                            nc.tensor.matmul(pp[:, a:a + bl], w_s[:],
                                             X[:, base + a:base + a + bl],
                                             start=True, stop=True)
                        sl = (slice(0, 64), slice(base, base + L3))
                        sh = (slice(64, 128), slice(base, base + L3))
                        if kind == "k":
                            # split k evacuations across DVE/ACT to keep DVE,
                            # the span-limiting engine, under ACT's load
                            if m <= 2:
                                nc.vector.tensor_copy(Kp[m][sl], pp[0:64, :L3])
                            if m >= 1:
                                nc.scalar.copy(Kp[m - 1][sh], pp[64:128, :L3])
                        elif kind == "v":
                            if m <= 2:
                                nc.scalar.copy(Vp[m][sl], pp[0:64, :L3])
                            if m >= 1:
                                nc.scalar.copy(Vp[m - 1][sh], pp[64:128, :L3])
                        elif m == 1:
                            nc.vector.tensor_copy(Q[sl], pp[0:64, :L3])
                        else:
                            nc.scalar.copy(Q[sh], pp[64:128, :L3])

        # ---- 27-neighbor softmax attention, PSUM-chunked over the plane
        accp = ctx.enter_context(tc.tile_pool(name="acc", bufs=1, space="PSUM"))
        OUTv = OUT.rearrange("p (r c) -> p r c", c=HP)
        GPSET = frozenset((0, 2, 6, 8, 9, 11, 15, 17, 18, 20, 21, 23, 24, 26))
        for ci, ((c0, L), (r0, r1)) in enumerate(zip(CHUNKS, OUT_ROWS)):
            den = accp.tile([128, 1536], f32, tag="den")
            num = accp.tile([128, 1536], f32, tag="num")
            for kv in range(27):
                kd, r = divmod(kv, 9)
                kh, kw = divmod(r, 3)
                dd = (kh - 1) * HP + (kw - 1)
                s_t = wpool.tile([128, 1536], f32, tag="s")
                nc.vector.scalar_tensor_tensor(
                    s_t[:, :L], Kp[kd][:, c0 + dd:c0 + dd + L],
                    b_s[:, kv:kv + 1], Q[:, c0:c0 + L], Alu.add, Alu.mult)
                e_t = wpool.tile([128, 1536], edt, tag="e")
                # bias keeps exp inside the ACT table range (softmax is
                # shift-invariant; the -28 cancels via the ln/exp normalize)
                nc.scalar.activation(e_t[:, :L], s_t[:, :L], Act.Exp, bias=ebias[:])
                ev_t = wpool.tile([128, 1536], edt, tag="ev")
                # DVE is the bottleneck engine; hand ~half the e*v products
                # to the otherwise-idle GPSIMD (stock Q7 tensor_tensor).
                ev_eng = nc.gpsimd if (kw == 1 or kv in GPSET) else nc.vector
                ev_eng.tensor_mul(ev_t[:, :L], e_t[:, :L],
                                  Vp[kd][:, c0 + dd:c0 + dd + L])
                st, sp = kv == 0, kv == 26
                for a, bl in _subs(L):
                    nc.tensor.matmul(den[:, a:a + bl], id_s[:], e_t[:, a:a + bl],
                                     start=st, stop=sp)
                    nc.tensor.matmul(num[:, a:a + bl], id_s[:], ev_t[:, a:a + bl],
                                     start=st, stop=sp)
            l_t = wpool.tile([128, 1536], f32, tag="s")
            nc.scalar.activation(l_t[:, :L], den[:, :L], Act.Ln)
            f_t = wpool.tile([128, 1536], f32, tag="f")
            nc.scalar.activation(f_t[:, :L], l_t[:, :L], Act.Exp, scale=-1.0)
            T = wpool.tile([128, 1536], f32, tag="t")
            nc.vector.tensor_mul(T[:, :L], num[:, :L], f_t[:, :L])
            # int8 quantize against this chunk's per-partition absmax; the
            # host divides by the exact same scale, so recip accuracy and the
            # 126 (vs 127) headroom only affect range, not correctness
            nc.vector.tensor_reduce(am_s[:, ci:ci + 1], T[:, :L],
                                    axis=mybir.AxisListType.X,
                                    op=Alu.max, apply_absolute_value=True)
            nc.vector.reciprocal(rc_s[:, ci:ci + 1], am_s[:, ci:ci + 1])
            nc.vector.tensor_scalar_mul(rc_s[:, ci:ci + 1],
                                        rc_s[:, ci:ci + 1], 126.0)
            nc.vector.tensor_scalar_mul(OUT[:, c0:c0 + L], T[:, :L],
                                        rc_s[:, ci:ci + 1])
            # rows fully covered by chunks <= this one stream out immediately
            nc.sync.dma_start(oc_d[:, r0 * 64:r1 * 64],
                              OUTv[:, 1 + r0:1 + r1, 1:65])
        nc.sync.dma_start(oc_d[:, 4096:4112], rc_s[:].bitcast(i8))
        # gather every core's packed result so one host fetch gets them all
        nc.gpsimd.collective_compute(
            "AllGather", mybir.AluOpType.bypass,
            replica_groups=[list(range(N_CORES))],
            ins=[oc_d[:]], outs=[og_d[:]])
        nc.sync.dma_start(out_d[:], og_d[:])
    nc.finalize()
    return nc


def _compile():
    """Build the Bass module once and cache a persistent jitted dispatcher.

    run_bass_kernel_spmd re-creates (and re-traces) its jit closure on every
    call; building it once here removes that per-call overhead and lets us
    keep the donated output buffers device-resident between calls.
    """
    import jax
    from concourse import mybir
    from concourse.bass2jax import (_bass_exec_p, partition_id_tensor,
                                    install_neuronx_cc_hook)
    from jax.sharding import Mesh, PartitionSpec, NamedSharding
    from jax.experimental.shard_map import shard_map

    install_neuronx_cc_hook()
    nc = _build()

    partition_name = nc.partition_id_tensor.name if nc.partition_id_tensor else None
    in_names, out_names, out_avals, zero_outs = [], [], [], []
    for alloc in nc.m.functions[0].allocations:
        if not isinstance(alloc, mybir.MemoryLocationSet):
            continue
        name = alloc.memorylocations[0].name
        if alloc.kind == "ExternalInput":
            if name != partition_name:
                in_names.append(name)
        elif alloc.kind == "ExternalOutput":
            shape = tuple(alloc.tensor_shape)
            dtype = mybir.dt.np(alloc.dtype)
            out_avals.append(jax.core.ShapedArray(shape, dtype))
            out_names.append(name)
            zero_outs.append(np.zeros((N_CORES * shape[0], *shape[1:]), dtype))
    n_params = len(in_names)
    n_outs = len(out_avals)
    in_names_full = list(in_names) + out_names
    if partition_name is not None:
        in_names_full.append(partition_name)
    donate = tuple(range(n_params, n_params + n_outs))

    def _body(*args):
        operands = list(args)
        if partition_name is not None:
            operands.append(partition_id_tensor())
        outs = _bass_exec_p.bind(
            *operands,
            out_avals=tuple(out_avals),
            in_names=tuple(in_names_full),
            out_names=tuple(out_names),
            lowering_input_output_aliases=(),
            sim_require_finite=True,
            sim_require_nnan=True,
            nc=nc,
        )
        return tuple(outs)

    devices = jax.devices()[:N_CORES]
    mesh = Mesh(np.asarray(devices), ("core",))
    in_specs = (PartitionSpec("core"),) * (n_params + n_outs)
    out_specs = (PartitionSpec("core"),) * n_outs
    fn = jax.jit(
        shard_map(_body, mesh=mesh, in_specs=in_specs, out_specs=out_specs,
                  check_rep=False),
        donate_argnums=donate,
        keep_unused=True,
    )
    _CACHE.update(nc=nc, fn=fn, in_names=in_names, prev_outs=zero_outs,
                  n_outs=n_outs,
                  shard=NamedSharding(mesh, PartitionSpec("core")))


def _aux_inputs(w_q, w_k, w_v, rel_d, rel_h, rel_w):
    """Weight-dependent per-core inputs (concatenated along axis 0)."""
    rd = np.asarray(rel_d, np.float32).reshape(21, 3)
    rh = np.asarray(rel_h, np.float32).reshape(21, 3)
    rw = np.asarray(rel_w, np.float32).reshape(22, 3)

    # one-hot halo selectors over the 16 gathered planes (gathered plane j =
    # padded depth plane j+1); left halo of core i = plane 2i -> j = 2i-1,
    # right halo = plane 2i+3 -> j = 2i+2; edge cores get all-zero rows.
    hs_g = np.zeros((N_CORES, 128, 16), np.float32)
    for i in range(N_CORES):
        if i > 0:
            hs_g[i, 0:64, 2 * i - 1] = 1.0
        if i < N_CORES - 1:
            hs_g[i, 64:128, 2 * i + 2] = 1.0

    kv27 = np.arange(27)
    kd_i, kh_i, kw_i = kv27 // 9, (kv27 // 3) % 3, kv27 % 3
    B64 = np.empty((64, 27), np.float32)
    B64[:21] = rd[:, kd_i]
    B64[21:42] = rh[:, kh_i]
    B64[42:] = rw[:, kw_i]
    B = np.concatenate([B64, B64], 0)

    import ml_dtypes
    idt = np.eye(128, dtype=np.float32)
    idt = idt.astype(ml_dtypes.bfloat16 if E_BF16 else np.float32)

    def dup(w):
        w2 = np.concatenate([w.T, w.T], 1).astype(np.float16)
        return np.tile(w2, (N_CORES, 1))

    return {
        "hsel": hs_g.reshape(N_CORES * 128, 16),
        "wk2": dup(np.asarray(w_k)),
        "wv2": dup(np.asarray(w_v)),
        "wq2": dup(np.asarray(w_q)),
        "bias": np.tile(B, (N_CORES, 1)),
        "ident": np.tile(idt, (N_CORES, 1)),
    }


def kernel(x, w_q, w_k, w_v, rel_d, rel_h, rel_w):
    import jax
    import hashlib

    if "fn" not in _CACHE:
        _compile()

    x = np.asarray(x, np.float32)
    # core i's own planes, partition = slot*64 + channel, unpadded
    bnd = np.asarray(x[0]).transpose(1, 0, 2, 3).astype(np.float16) \
        .reshape(N_CORES * 128, 4096)

    # weights/bias/ident/hsel are tiny but cost per-shard transfer overhead;
    # keep them device-resident across calls, re-uploading only if changed
    h = hashlib.sha1()
    for a in (w_q, w_k, w_v, rel_d, rel_h, rel_w):
        h.update(np.ascontiguousarray(a).tobytes())
    key = h.hexdigest()
    if _CACHE.get("aux_key") != key:
        aux = _aux_inputs(w_q, w_k, w_v, rel_d, rel_h, rel_w)
        _CACHE["aux_dev"] = {k: jax.device_put(v, _CACHE["shard"])
                             for k, v in aux.items()}
        _CACHE["aux_key"] = key
    gmaps = dict(_CACHE["aux_dev"])
    gmaps["bnd"] = bnd

    args = [gmaps[nm] for nm in _CACHE["in_names"]]
    out_arrs = _CACHE["fn"](*args, *_CACHE["prev_outs"])
    # recycle the device-resident output buffers as next call's donation args
    # (their contents are irrelevant: the NEFF writes every output element)
    _CACHE["prev_outs"] = list(out_arrs)

    # every core holds the full gathered result; fetch only device 0's shard
    o = np.asarray(out_arrs[0].addressable_shards[0].data)  # [8*128, 4112] i8
    sc = np.ascontiguousarray(o[:, 4096:4112]).view(np.float32)  # [8*128, 4]
    o8 = o[:, :4096].reshape(N_CORES * 128, 64, 64)
    band = np.empty(64, np.int64)
    for ci, (r0, r1) in enumerate(OUT_ROWS):
        band[r0:r1] = ci
    rsc = np.float32(1.0) / sc[:, band]
    of = np.multiply(o8, rsc[:, :, None], dtype=np.float32)
    out = np.empty((1, 64, 16, 64, 64), np.float32)
    out.reshape(64, N_CORES, 2, 64, 64)[:] = \
        of.reshape(N_CORES, 2, 64, 64, 64).transpose(2, 0, 1, 3, 4)
    return out


# revision 26
# speedup vs baseline: 1.6599x; 1.6599x over previous
"""AttentionConv3D Trainium2 kernel.

Computation (per channel c, voxel (d,h,w)):
    q,k,v = 1x1x1 convs of x;  s_kv = q * (k_pad[nbr kv] + rel_bias(c,kv))
    out   = sum_kv softmax_kv(s) * v_pad[nbr kv]         (27 = 3x3x3 window)

Strategy: depth-shard over 8 cores (2 output depth planes each, 1-plane halo).
On-device layout: partition p = dl*64 + c (dl in {0,1} local depth), free dim
= zero-padded 66x66 plane (4356). Per kv-neighbor the window access is a
free-dim offset (kh-1)*66 + (kw-1) into one of three depth-plane buffers
K[kd]; the rel bias collapses to a per-partition scalar B[p, kv], so
s = (K_shift + B)*q is ONE DVE scalar_tensor_tensor op. exp on ACT;
numerator/denominator accumulated with identity matmuls into PSUM on the
TensorEngine; 1/den via exp(-ln(den)) on ACT.

The wall clock is dominated by the ~40MB/s (half-duplex) axon tunnel, so I/O
is minimized:
 - each core uploads ONLY its two unpadded fp16 depth planes (8.4MB total,
   no halo duplication, padding assembled on device); the 1-plane halos are
   exchanged on device: a world AllGather of every core's plane pair, then a
   per-core one-hot masked sum (host-uploaded selection scalars, 16 DVE
   select-accumulate ops) picks the two neighbor planes — edge cores get
   all-zero masks, i.e. free zero padding.
 - projection matmuls run fp16 x fp16 -> fp32 PSUM.
 - the output is quantized on device to int8 against a per-(partition, row
   band) absmax scale (the host divides by the exact same downloaded scale),
   AllGathered across cores, and fetched as ONE 4.3MB transfer from core 0's
   shard instead of 16 per-shard fetches.
 - weights/bias/identity/halo-selectors are kept device-resident across
   calls (sha1 of the weight args guards staleness).
 - donated output buffers are device-resident (recycled between calls).
 - the jitted dispatch closure is built once and cached.
"""

import sys
import numpy as np

for _p in ("/opt/trn_rl_repo", "/root/.axon_site/_ro/trn_rl_repo"):
    if _p not in sys.path:
        sys.path.insert(0, _p)

HP = 66               # padded plane edge
HW = HP * HP          # 4356
CHUNKS = [(67, 1386), (1453, 1452), (2905, 1384)]  # covers [67, 4289), chunk
# edges row-aligned so each output row band is quantized by exactly one
# chunk's int8 scale. Chunk 0's window reads ([0, 1520)) still fit inside proj
# col-chunk 0 so the kv loop overlaps the tail of the projection phase.
PROJ = [(0, 1536), (1536, 1536), (3072, 1284)]     # proj psum chunks over 4356
OUT_ROWS = [(0, 21), (21, 43), (43, 64)]           # row bands DMA'd per chunk
ZW = 512              # halo-assembly column chunk (8 chunks cover 4096)

# hot-path dtype knobs (fp32 = safe; bf16 halves DVE cost of the e*v path)
E_BF16 = True   # e / v / ev tiles + identity in bf16 (PE still accums fp32)

N_CORES = 8

_CACHE = {}


def _subs(L):
    return [(0, 512), (512, 512), (1024, L - 1024)]


def _build():
    from contextlib import ExitStack
    import concourse.bacc as bacc
    import concourse.tile as tile
    from concourse import mybir

    f32 = mybir.dt.float32
    f16 = mybir.dt.float16
    bf16 = mybir.dt.bfloat16
    edt = bf16 if E_BF16 else f32
    Alu = mybir.AluOpType
    Act = mybir.ActivationFunctionType

    nc = bacc.Bacc("TRN2", target_bir_lowering=False)
    # own two unpadded depth planes, partition = slot*64 + channel
    bnd_d = nc.dram_tensor("bnd", [128, 4096], f16, kind="ExternalInput")
    # halo selection scalars: rows 0:64 pick the left-halo plane, 64:128 the
    # right-halo plane, as one-hot over the 16 gathered planes
    hs_d = nc.dram_tensor("hsel", [128, 16], f32, kind="ExternalInput")
    wk_d = nc.dram_tensor("wk2", [64, 128], f16, kind="ExternalInput")
    wv_d = nc.dram_tensor("wv2", [64, 128], f16, kind="ExternalInput")
    wq_d = nc.dram_tensor("wq2", [64, 128], f16, kind="ExternalInput")
    b_d = nc.dram_tensor("bias", [128, 27], f32, kind="ExternalInput")
    id_d = nc.dram_tensor("ident", [128, 128], edt, kind="ExternalInput")
    i8 = mybir.dt.int8
    # every core receives the full gathered result; the host fetches only
    # core 0's copy (one transfer instead of 16 per-shard fetches)
    out_d = nc.dram_tensor("out", [N_CORES * 128, 4112], i8,
                           kind="ExternalOutput")

    # collective staging (collectives can't touch I/O tensors directly)
    bb_d = nc.dram_tensor("bb", [128, 4096], f16)
    g_d = nc.dram_tensor("g", [16, 64, 4096], f16, addr_space="Shared")
    # per-core packed result: 64*64 int8 voxels + 16 bytes of f32 scales
    oc_d = nc.dram_tensor("oc", [128, 4112], i8)
    og_d = nc.dram_tensor("og", [N_CORES * 128, 4112], i8, addr_space="Shared")

    with tile.TileContext(nc) as tc, ExitStack() as ctx:
        singles = ctx.enter_context(tc.tile_pool(name="singles", bufs=1))
        planes = ctx.enter_context(tc.tile_pool(name="planes", bufs=1))
        wpool = ctx.enter_context(tc.tile_pool(name="work", bufs=2))

        wk_s = singles.tile([64, 128], f16, tag="wk")
        wv_s = singles.tile([64, 128], f16, tag="wv")
        wq_s = singles.tile([64, 128], f16, tag="wq")
        id_s = singles.tile([128, 128], edt, tag="id")
        b_s = singles.tile([128, 27], f32, tag="b")
        hs_s = singles.tile([128, 16], f32, tag="hs")
        ebias = singles.tile([128, 1], f32, tag="ebias")
        nc.vector.memset(ebias[:], -28.0)
        for t, d in ((wk_s, wk_d), (wv_s, wv_d), (wq_s, wq_d),
                     (id_s, id_d), (b_s, b_d), (hs_s, hs_d)):
            nc.sync.dma_start(t[:], d[:])

        # ---- halo exchange: world AllGather of everyone's plane pair
        nc.gpsimd.dma_start(bb_d[:], bnd_d[:])
        nc.gpsimd.collective_compute(
            "AllGather", mybir.AluOpType.bypass,
            replica_groups=[list(range(N_CORES))],
            ins=[bb_d[:]], outs=[g_d[:]])

        # the four padded depth planes, assembled from unpadded data: memset
        # zeroes the 66x66 borders, the interiors land via strided DMAs
        XP = [planes.tile([64, HW], f16, tag=f"xp{m}", name=f"xp{m}")
              for m in range(4)]
        XPv = [t.rearrange("p (r c) -> p r c", c=HP) for t in XP]
        for t in XP:
            nc.vector.memset(t[:], 0.0)
        bnd_v = bnd_d[:].rearrange("p (r c) -> p r c", c=64)
        nc.sync.dma_start(XPv[1][:, 1:65, 1:65], bnd_v[0:64])
        nc.sync.dma_start(XPv[2][:, 1:65, 1:65], bnd_v[64:128])

        with tc.tile_pool(name="gt", bufs=1) as gpool:
            # 0:64 = left halo plane, 64:128 = right halo plane (unpadded)
            XH = gpool.tile([128, 4096], f16, tag="xh")
            for w in range(8):
                ws = w * ZW
                GT = gpool.tile([128, 16 * ZW], f16, tag="gt")
                GTv = GT.rearrange("p (j z) -> p j z", j=16)
                src = g_d[:, :, ws:ws + ZW].transpose([1, 0, 2])
                nc.sync.dma_start(GTv[0:64, :, :], src)
                nc.sync.dma_start(GTv[64:128, :, :], src)
                # one-hot select-accumulate over the 16 gathered planes
                nc.vector.tensor_scalar_mul(
                    XH[:, ws:ws + ZW], GT[:, 0:ZW], hs_s[:, 0:1])
                for j in range(1, 16):
                    nc.vector.scalar_tensor_tensor(
                        XH[:, ws:ws + ZW], GT[:, j * ZW:(j + 1) * ZW],
                        hs_s[:, j:j + 1], XH[:, ws:ws + ZW],
                        Alu.mult, Alu.add)
            # pad the halo planes (and bring the right halo down to base
            # partition 0, which matmul moving operands require)
            XHv = XH.rearrange("p (r c) -> p r c", c=64)
            nc.sync.dma_start(XPv[0][:, 1:65, 1:65], XHv[0:64])
            nc.sync.dma_start(XPv[3][:, 1:65, 1:65], XHv[64:128])

        Kp = [planes.tile([128, HW], f32, tag=f"k{i}", name=f"k{i}") for i in range(3)]
        Vp = [planes.tile([128, HW], edt, tag=f"v{i}", name=f"v{i}") for i in range(3)]
        Q = planes.tile([128, HW], f32, tag="q")
        OUT = planes.tile([128, HW], i8, tag="o")
        # int8 quantization state: per-chunk per-partition scale = 126/absmax
        am_s = singles.tile([128, 4], f32, tag="am")
        rc_s = singles.tile([128, 4], f32, tag="rc")
        nc.vector.memset(rc_s[:], 1.0)

        # ---- projections: plane m -> k/v (dual-copy weights give the same
        # output plane on partitions 0:64 and 64:128), q for m in {1,2}.
        # column-chunk OUTER so all planes' first 1536 columns (what kv chunk 0
        # needs) are projected before any plane's later columns.
        Xsrc = [t[:] for t in XP]
        with tc.tile_pool(name="pp", bufs=2, space="PSUM") as ppool:
            for base, L3 in PROJ:
                for m in range(4):
                    X = Xsrc[m]
                    projs = [(wk_s, "k"), (wv_s, "v")]
                    if m in (1, 2):
                        projs.append((wq_s, "q"))
                    for w_s, kind in projs:
                        pp = ppool.tile([128, 1536], f32, tag="pp")
                        for a, bl in _subs(L3):
                            nc.tensor.matmul(pp[:, a:a + bl], w_s[:],
                                             X[:, base + a:base + a + bl],
                                             start=True, stop=True)
                        sl = (slice(0, 64), slice(base, base + L3))
                        sh = (slice(64, 128), slice(base, base + L3))
                        if kind == "k":
                            # split k evacuations across DVE/ACT to keep DVE,
                            # the span-limiting engine, under ACT's load
                            if m <= 2:
                                nc.vector.tensor_copy(Kp[m][sl], pp[0:64, :L3])
                            if m >= 1:
                                nc.scalar.copy(Kp[m - 1][sh], pp[64:128, :L3])
                        elif kind == "v":
                            if m <= 2:
                                nc.scalar.copy(Vp[m][sl], pp[0:64, :L3])
                            if m >= 1:
                                nc.scalar.copy(Vp[m - 1][sh], pp[64:128, :L3])
                        elif m == 1:
                            nc.vector.tensor_copy(Q[sl], pp[0:64, :L3])
                        else:
                            nc.scalar.copy(Q[sh], pp[64:128, :L3])

        # ---- 27-neighbor softmax attention, PSUM-chunked over the plane
        accp = ctx.enter_context(tc.tile_pool(name="acc", bufs=1, space="PSUM"))
        OUTv = OUT.rearrange("p (r c) -> p r c", c=HP)
        GPSET = frozenset((0, 2, 6, 8, 9, 11, 15, 17, 18, 20, 21, 23, 24, 26))
        for ci, ((c0, L), (r0, r1)) in enumerate(zip(CHUNKS, OUT_ROWS)):
            den = accp.tile([128, 1536], f32, tag="den")
            num = accp.tile([128, 1536], f32, tag="num")
            for kv in range(27):
                kd, r = divmod(kv, 9)
                kh, kw = divmod(r, 3)
                dd = (kh - 1) * HP + (kw - 1)
                s_t = wpool.tile([128, 1536], f32, tag="s")
                nc.vector.scalar_tensor_tensor(
                    s_t[:, :L], Kp[kd][:, c0 + dd:c0 + dd + L],
                    b_s[:, kv:kv + 1], Q[:, c0:c0 + L], Alu.add, Alu.mult)
                e_t = wpool.tile([128, 1536], edt, tag="e")
                # bias keeps exp inside the ACT table range (softmax is
                # shift-invariant; the -28 cancels via the ln/exp normalize)
                nc.scalar.activation(e_t[:, :L], s_t[:, :L], Act.Exp, bias=ebias[:])
                ev_t = wpool.tile([128, 1536], edt, tag="ev")
                # DVE is the bottleneck engine; hand ~half the e*v products
                # to the otherwise-idle GPSIMD (stock Q7 tensor_tensor).
                ev_eng = nc.gpsimd if (kw == 1 or kv in GPSET) else nc.vector
                ev_eng.tensor_mul(ev_t[:, :L], e_t[:, :L],
                                  Vp[kd][:, c0 + dd:c0 + dd + L])
                st, sp = kv == 0, kv == 26
                for a, bl in _subs(L):
                    nc.tensor.matmul(den[:, a:a + bl], id_s[:], e_t[:, a:a + bl],
                                     start=st, stop=sp)
                    nc.tensor.matmul(num[:, a:a + bl], id_s[:], ev_t[:, a:a + bl],
                                     start=st, stop=sp)
            l_t = wpool.tile([128, 1536], f32, tag="s")
            nc.scalar.activation(l_t[:, :L], den[:, :L], Act.Ln)
            f_t = wpool.tile([128, 1536], f32, tag="f")
            nc.scalar.activation(f_t[:, :L], l_t[:, :L], Act.Exp, scale=-1.0)
            T = wpool.tile([128, 1536], f32, tag="t")
            nc.vector.tensor_mul(T[:, :L], num[:, :L], f_t[:, :L])
            # int8 quantize against this chunk's per-partition absmax; the
            # host divides by the exact same scale, so recip accuracy and the
            # 126 (vs 127) headroom only affect range, not correctness
            nc.vector.tensor_reduce(am_s[:, ci:ci + 1], T[:, :L],
                                    axis=mybir.AxisListType.X,
                                    op=Alu.max, apply_absolute_value=True)
            nc.vector.reciprocal(rc_s[:, ci:ci + 1], am_s[:, ci:ci + 1])
            nc.vector.tensor_scalar_mul(rc_s[:, ci:ci + 1],
                                        rc_s[:, ci:ci + 1], 126.0)
            nc.vector.tensor_scalar_mul(OUT[:, c0:c0 + L], T[:, :L],
                                        rc_s[:, ci:ci + 1])
            # rows fully covered by chunks <= this one stream out immediately
            nc.sync.dma_start(oc_d[:, r0 * 64:r1 * 64],
                              OUTv[:, 1 + r0:1 + r1, 1:65])
        nc.sync.dma_start(oc_d[:, 4096:4112], rc_s[:].bitcast(i8))
        # gather every core's packed result so one host fetch gets them all
        nc.gpsimd.collective_compute(
            "AllGather", mybir.AluOpType.bypass,
            replica_groups=[list(range(N_CORES))],
            ins=[oc_d[:]], outs=[og_d[:]])
        nc.sync.dma_start(out_d[:], og_d[:])
    nc.finalize()
    return nc


def _compile():
    """Build the Bass module once and cache a persistent jitted dispatcher.

    run_bass_kernel_spmd re-creates (and re-traces) its jit closure on every
    call; building it once here removes that per-call overhead and lets us
    keep the donated output buffers device-resident between calls.
    """
    import jax
    from concourse import mybir
    from concourse.bass2jax import (_bass_exec_p, partition_id_tensor,
                                    install_neuronx_cc_hook)
    from jax.sharding import Mesh, PartitionSpec, NamedSharding
    from jax.experimental.shard_map import shard_map

    install_neuronx_cc_hook()
    nc = _build()

    partition_name = nc.partition_id_tensor.name if nc.partition_id_tensor else None
    in_names, out_names, out_avals, zero_outs = [], [], [], []
    for alloc in nc.m.functions[0].allocations:
        if not isinstance(alloc, mybir.MemoryLocationSet):
            continue
        name = alloc.memorylocations[0].name
        if alloc.kind == "ExternalInput":
            if name != partition_name:
                in_names.append(name)
        elif alloc.kind == "ExternalOutput":
            shape = tuple(alloc.tensor_shape)
            dtype = mybir.dt.np(alloc.dtype)
            out_avals.append(jax.core.ShapedArray(shape, dtype))
            out_names.append(name)
            zero_outs.append(np.zeros((N_CORES * shape[0], *shape[1:]), dtype))
    n_params = len(in_names)
    n_outs = len(out_avals)
    in_names_full = list(in_names) + out_names
    if partition_name is not None:
        in_names_full.append(partition_name)
    donate = tuple(range(n_params, n_params + n_outs))

    def _body(*args):
        operands = list(args)
        if partition_name is not None:
            operands.append(partition_id_tensor())
        outs = _bass_exec_p.bind(
            *operands,
            out_avals=tuple(out_avals),
            in_names=tuple(in_names_full),
            out_names=tuple(out_names),
            lowering_input_output_aliases=(),
            sim_require_finite=True,
            sim_require_nnan=True,
            nc=nc,
        )
        return tuple(outs)

    devices = jax.devices()[:N_CORES]
    mesh = Mesh(np.asarray(devices), ("core",))
    in_specs = (PartitionSpec("core"),) * (n_params + n_outs)
    out_specs = (PartitionSpec("core"),) * n_outs
    fn = jax.jit(
        shard_map(_body, mesh=mesh, in_specs=in_specs, out_specs=out_specs,
                  check_rep=False),
        donate_argnums=donate,
        keep_unused=True,
    )
    shard = NamedSharding(mesh, PartitionSpec("core"))
    # device-resident from the start so every call (including the first) hits
    # the same jit specialization (np donation args would retrace)
    dev_zeros = [jax.device_put(z, shard) for z in zero_outs]
    _CACHE.update(nc=nc, fn=fn, in_names=in_names, prev_outs=dev_zeros,
                  n_outs=n_outs, shard=shard)


def _aux_inputs(w_q, w_k, w_v, rel_d, rel_h, rel_w):
    """Weight-dependent per-core inputs (concatenated along axis 0)."""
    rd = np.asarray(rel_d, np.float32).reshape(21, 3)
    rh = np.asarray(rel_h, np.float32).reshape(21, 3)
    rw = np.asarray(rel_w, np.float32).reshape(22, 3)

    # one-hot halo selectors over the 16 gathered planes (gathered plane j =
    # padded depth plane j+1); left halo of core i = plane 2i -> j = 2i-1,
    # right halo = plane 2i+3 -> j = 2i+2; edge cores get all-zero rows.
    hs_g = np.zeros((N_CORES, 128, 16), np.float32)
    for i in range(N_CORES):
        if i > 0:
            hs_g[i, 0:64, 2 * i - 1] = 1.0
        if i < N_CORES - 1:
            hs_g[i, 64:128, 2 * i + 2] = 1.0

    kv27 = np.arange(27)
    kd_i, kh_i, kw_i = kv27 // 9, (kv27 // 3) % 3, kv27 % 3
    B64 = np.empty((64, 27), np.float32)
    B64[:21] = rd[:, kd_i]
    B64[21:42] = rh[:, kh_i]
    B64[42:] = rw[:, kw_i]
    B = np.concatenate([B64, B64], 0)

    import ml_dtypes
    idt = np.eye(128, dtype=np.float32)
    idt = idt.astype(ml_dtypes.bfloat16 if E_BF16 else np.float32)

    def dup(w):
        w2 = np.concatenate([w.T, w.T], 1).astype(np.float16)
        return np.tile(w2, (N_CORES, 1))

    return {
        "hsel": hs_g.reshape(N_CORES * 128, 16),
        "wk2": dup(np.asarray(w_k)),
        "wv2": dup(np.asarray(w_v)),
        "wq2": dup(np.asarray(w_q)),
        "bias": np.tile(B, (N_CORES, 1)),
        "ident": np.tile(idt, (N_CORES, 1)),
    }


def kernel(x, w_q, w_k, w_v, rel_d, rel_h, rel_w):
    import jax
    import hashlib

    if "fn" not in _CACHE:
        _compile()

    x = np.asarray(x, np.float32)
    # core i's own planes, partition = slot*64 + channel, unpadded
    bnd = np.asarray(x[0]).transpose(1, 0, 2, 3).astype(np.float16) \
        .reshape(N_CORES * 128, 4096)

    # weights/bias/ident/hsel are tiny but cost per-shard transfer overhead;
    # keep them device-resident across calls, re-uploading only if changed
    h = hashlib.sha1()
    for a in (w_q, w_k, w_v, rel_d, rel_h, rel_w):
        h.update(np.ascontiguousarray(a).tobytes())
    key = h.hexdigest()
    if _CACHE.get("aux_key") != key:
        aux = _aux_inputs(w_q, w_k, w_v, rel_d, rel_h, rel_w)
        _CACHE["aux_dev"] = {k: jax.device_put(v, _CACHE["shard"])
                             for k, v in aux.items()}
        _CACHE["aux_key"] = key
    gmaps = dict(_CACHE["aux_dev"])
    gmaps["bnd"] = bnd

    args = [gmaps[nm] for nm in _CACHE["in_names"]]
    out_arrs = _CACHE["fn"](*args, *_CACHE["prev_outs"])
    # recycle the device-resident output buffers as next call's donation args
    # (their contents are irrelevant: the NEFF writes every output element)
    _CACHE["prev_outs"] = list(out_arrs)

    # every core holds the full gathered result; fetch only device 0's shard
    o = np.asarray(out_arrs[0].addressable_shards[0].data)  # [8*128, 4112] i8
    sc = np.ascontiguousarray(o[:, 4096:4112]).view(np.float32)  # [8*128, 4]
    o8 = o[:, :4096].reshape(N_CORES * 128, 64, 64)
    band = np.empty(64, np.int64)
    for ci, (r0, r1) in enumerate(OUT_ROWS):
        band[r0:r1] = ci
    rsc = np.float32(1.0) / sc[:, band]
    of = np.multiply(o8, rsc[:, :, None], dtype=np.float32)
    out = np.empty((1, 64, 16, 64, 64), np.float32)
    out.reshape(64, N_CORES, 2, 64, 64)[:] = \
        of.reshape(N_CORES, 2, 64, 64, 64).transpose(2, 0, 1, 3, 4)
    return out


# revision 32
# speedup vs baseline: 1.7518x; 1.0553x over previous
"""AttentionConv3D Trainium2 kernel.

Computation (per channel c, voxel (d,h,w)):
    q,k,v = 1x1x1 convs of x;  s_kv = q * (k_pad[nbr kv] + rel_bias(c,kv))
    out   = sum_kv softmax_kv(s) * v_pad[nbr kv]         (27 = 3x3x3 window)

Strategy: depth-shard over 8 cores (2 output depth planes each, 1-plane halo).
On-device layout: partition p = dl*64 + c (dl in {0,1} local depth), free dim
= zero-padded 66x66 plane (4356). Per kv-neighbor the window access is a
free-dim offset (kh-1)*66 + (kw-1) into one of three depth-plane buffers
K[kd]; the rel bias collapses to a per-partition scalar B[p, kv], so
s = (K_shift + B)*q is ONE DVE scalar_tensor_tensor op. exp on ACT;
numerator/denominator accumulated with identity matmuls into PSUM on the
TensorEngine; 1/den via exp(-ln(den)) on ACT.

The wall clock is dominated by the ~40MB/s (half-duplex) axon tunnel, so I/O
is minimized:
 - each core uploads ONLY its two unpadded fp16 depth planes (8.4MB total,
   no halo duplication, padding assembled on device); the 1-plane halos are
   exchanged on device: a world AllGather of every core's plane pair, then a
   per-core one-hot masked sum (host-uploaded selection scalars, 16 DVE
   select-accumulate ops) picks the two neighbor planes — edge cores get
   all-zero masks, i.e. free zero padding.
 - projection matmuls run fp16 x fp16 -> fp32 PSUM.
 - the output is quantized on device to int8 against a per-(partition, row
   band) absmax scale (the host divides by the exact same downloaded scale),
   AllGathered across cores, and fetched as ONE 4.3MB transfer from core 0's
   shard instead of 16 per-shard fetches.
 - weights/bias/identity/halo-selectors are kept device-resident across
   calls (sha1 of the weight args guards staleness).
 - donated output buffers are device-resident (recycled between calls).
 - the jitted dispatch closure is built once and cached.
"""

import sys
import numpy as np

for _p in ("/opt/trn_rl_repo", "/root/.axon_site/_ro/trn_rl_repo"):
    if _p not in sys.path:
        sys.path.insert(0, _p)

HP = 66               # padded plane edge
HW = HP * HP          # 4356
CHUNKS = [(67, 1386), (1453, 1452), (2905, 1384)]  # covers [67, 4289), chunk
# edges row-aligned so each output row band is quantized by exactly one
# chunk's int8 scale. Chunk 0's window reads ([0, 1520)) still fit inside proj
# col-chunk 0 so the kv loop overlaps the tail of the projection phase.
PROJ = [(0, 1536), (1536, 1536), (3072, 1284)]     # proj psum chunks over 4356
OUT_ROWS = [(0, 21), (21, 43), (43, 64)]           # row bands DMA'd per chunk
ZW = 512              # halo-assembly column chunk (8 chunks cover 4096)

# hot-path dtype knobs (fp32 = safe; bf16 halves DVE cost of the e*v path)
E_BF16 = True   # e / v / ev tiles + identity in bf16 (PE still accums fp32)

N_CORES = 8

_CACHE = {}

from concurrent.futures import ThreadPoolExecutor
_POOL = ThreadPoolExecutor(8)


def _subs(L):
    return [(0, 512), (512, 512), (1024, L - 1024)]


def _build():
    from contextlib import ExitStack
    import concourse.bacc as bacc
    import concourse.tile as tile
    from concourse import mybir

    f32 = mybir.dt.float32
    f16 = mybir.dt.float16
    bf16 = mybir.dt.bfloat16
    edt = bf16 if E_BF16 else f32
    Alu = mybir.AluOpType
    Act = mybir.ActivationFunctionType

    nc = bacc.Bacc("TRN2", target_bir_lowering=False)
    u8 = mybir.dt.uint8
    # own two unpadded depth planes, partition = slot*64 + channel, packed as
    # 12-bit fixed point (2 values per 3 bytes) + 4 trailing bytes holding the
    # per-partition f32 dequant range (absmax)
    bnd_d = nc.dram_tensor("bnd", [128, 6148], u8, kind="ExternalInput")
    # halo selection scalars: rows 0:64 pick the left-halo plane, 64:128 the
    # right-halo plane, as one-hot over the 16 gathered planes
    hs_d = nc.dram_tensor("hsel", [128, 16], f32, kind="ExternalInput")
    wk_d = nc.dram_tensor("wk2", [64, 128], f16, kind="ExternalInput")
    wv_d = nc.dram_tensor("wv2", [64, 128], f16, kind="ExternalInput")
    wq_d = nc.dram_tensor("wq2", [64, 128], f16, kind="ExternalInput")
    b_d = nc.dram_tensor("bias", [128, 27], f32, kind="ExternalInput")
    id_d = nc.dram_tensor("ident", [128, 128], edt, kind="ExternalInput")
    i8 = mybir.dt.int8
    # every core receives the full gathered result; the host fetches only
    # core 0's copy (one transfer instead of 16 per-shard fetches)
    out_d = nc.dram_tensor("out", [N_CORES * 128, 4112], i8,
                           kind="ExternalOutput")

    # collective staging (collectives can't touch I/O tensors directly)
    bb_d = nc.dram_tensor("bb", [128, 4096], f16)
    g_d = nc.dram_tensor("g", [16, 64, 4096], f16, addr_space="Shared")
    # per-core packed result: 64*64 int8 voxels + 16 bytes of f32 scales
    oc_d = nc.dram_tensor("oc", [128, 4112], i8)
    og_d = nc.dram_tensor("og", [N_CORES * 128, 4112], i8, addr_space="Shared")

    with tile.TileContext(nc) as tc, ExitStack() as ctx:
        singles = ctx.enter_context(tc.tile_pool(name="singles", bufs=1))
        planes = ctx.enter_context(tc.tile_pool(name="planes", bufs=1))
        wpool = ctx.enter_context(tc.tile_pool(name="work", bufs=2))

        wk_s = singles.tile([64, 128], f16, tag="wk")
        wv_s = singles.tile([64, 128], f16, tag="wv")
        wq_s = singles.tile([64, 128], f16, tag="wq")
        id_s = singles.tile([128, 128], edt, tag="id")
        b_s = singles.tile([128, 27], f32, tag="b")
        hs_s = singles.tile([128, 16], f32, tag="hs")
        ebias = singles.tile([128, 1], f32, tag="ebias")
        nc.vector.memset(ebias[:], -28.0)
        for t, d in ((wk_s, wk_d), (wv_s, wv_d), (wq_s, wq_d),
                     (id_s, id_d), (b_s, b_d), (hs_s, hs_d)):
            nc.sync.dma_start(t[:], d[:])

        # the four padded depth planes, assembled from unpadded data: memset
        # zeroes the 66x66 borders, the interiors land via strided DMAs
        XP = [planes.tile([64, HW], f16, tag=f"xp{m}", name=f"xp{m}")
              for m in range(4)]
        XPv = [t.rearrange("p (r c) -> p r c", c=HP) for t in XP]
        for t in XP:
            nc.vector.memset(t[:], 0.0)

        # ---- unpack the 12-bit planes to fp16 (float-domain bit math is
        # exact for 12-bit ints), then halo-exchange the fp16 result
        with tc.tile_pool(name="up", bufs=1) as upool:
            XU = upool.tile([128, 4096], f16, tag="xu")
            XUv = XU.rearrange("p (n t) -> p n t", t=2)
            U8s = upool.tile([128, 4], u8, tag="u8s")
            nc.sync.dma_start(U8s[:], bnd_d[:, 6144:6148])
            m_s = singles.tile([128, 1], f32, tag="m")
            mb_s = singles.tile([128, 1], f32, tag="mb")
            nc.vector.tensor_scalar_mul(m_s[:], U8s[:].bitcast(f32),
                                        1.0 / 2047.0)
            nc.vector.tensor_scalar_mul(mb_s[:], m_s[:], -2048.0)
            for c in range(4):
                U8 = upool.tile([128, 1536], u8, tag="u8")
                nc.sync.dma_start(U8[:], bnd_d[:, c * 1536:(c + 1) * 1536])
                U3 = U8[:].rearrange("p (n t) -> p n t", t=3)
                b0f = upool.tile([128, 512], f32, tag="b0")
                b1f = upool.tile([128, 512], f32, tag="b1")
                b2f = upool.tile([128, 512], f32, tag="b2")
                nc.scalar.copy(b0f[:], U3[:, :, 0])
                nc.scalar.copy(b1f[:], U3[:, :, 1])
                nc.scalar.copy(b2f[:], U3[:, :, 2])
                # hi = b1 >> 4 without mod (invalid on TensorScalar): the
                # offset -15/32 makes round-to-nearest(b1/16 - 0.46875) hit
                # floor(b1/16) exactly for every 4-bit fraction (never a tie)
                t_f = upool.tile([128, 512], f32, tag="t")
                nc.vector.tensor_scalar(t_f[:], b1f[:], 0.0625, -0.46875,
                                        Alu.mult, Alu.add)
                hi_i = upool.tile([128, 512], mybir.dt.int16, tag="hi")
                nc.vector.tensor_copy(hi_i[:], t_f[:])   # f32->i16 rounds
                hi_f = upool.tile([128, 512], f32, tag="hf")
                nc.vector.tensor_copy(hi_f[:], hi_i[:])
                # v0 = (b0 + 256*b1) - 4096*hi;  v1 = hi + 16*b2
                v0 = upool.tile([128, 512], f32, tag="v0")
                v1 = upool.tile([128, 512], f32, tag="v1")
                nc.vector.scalar_tensor_tensor(v0[:], b1f[:], 256.0, b0f[:],
                                               Alu.mult, Alu.add)
                nc.vector.scalar_tensor_tensor(v0[:], hi_f[:], -4096.0, v0[:],
                                               Alu.mult, Alu.add)
                nc.vector.scalar_tensor_tensor(v1[:], b2f[:], 16.0, hi_f[:],
                                               Alu.mult, Alu.add)
                sl = slice(c * 512, (c + 1) * 512)
                nc.vector.tensor_scalar(XUv[:, sl, 0], v0[:], m_s[:], mb_s[:],
                                        Alu.mult, Alu.add)
                nc.vector.tensor_scalar(XUv[:, sl, 1], v1[:], m_s[:], mb_s[:],
                                        Alu.mult, Alu.add)
            # own-plane interiors + the collective bounce, straight from XU
            XUr = XU.rearrange("p (r c) -> p r c", c=64)
            nc.sync.dma_start(XPv[1][:, 1:65, 1:65], XUr[0:64])
            nc.sync.dma_start(XPv[2][:, 1:65, 1:65], XUr[64:128])
            nc.gpsimd.dma_start(bb_d[:], XU[:])

        # ---- halo exchange: world AllGather of everyone's plane pair
        nc.gpsimd.collective_compute(
            "AllGather", mybir.AluOpType.bypass,
            replica_groups=[list(range(N_CORES))],
            ins=[bb_d[:]], outs=[g_d[:]])

        with tc.tile_pool(name="gt", bufs=1) as gpool:
            # 0:64 = left halo plane, 64:128 = right halo plane (unpadded)
            XH = gpool.tile([128, 4096], f16, tag="xh")
            for w in range(8):
                ws = w * ZW
                GT = gpool.tile([128, 16 * ZW], f16, tag="gt")
                GTv = GT.rearrange("p (j z) -> p j z", j=16)
                src = g_d[:, :, ws:ws + ZW].transpose([1, 0, 2])
                nc.sync.dma_start(GTv[0:64, :, :], src)
                nc.sync.dma_start(GTv[64:128, :, :], src)
                # one-hot select-accumulate over the 16 gathered planes
                nc.vector.tensor_scalar_mul(
                    XH[:, ws:ws + ZW], GT[:, 0:ZW], hs_s[:, 0:1])
                for j in range(1, 16):
                    nc.vector.scalar_tensor_tensor(
                        XH[:, ws:ws + ZW], GT[:, j * ZW:(j + 1) * ZW],
                        hs_s[:, j:j + 1], XH[:, ws:ws + ZW],
                        Alu.mult, Alu.add)
            # pad the halo planes (and bring the right halo down to base
            # partition 0, which matmul moving operands require)
            XHv = XH.rearrange("p (r c) -> p r c", c=64)
            nc.sync.dma_start(XPv[0][:, 1:65, 1:65], XHv[0:64])
            nc.sync.dma_start(XPv[3][:, 1:65, 1:65], XHv[64:128])

        Kp = [planes.tile([128, HW], f32, tag=f"k{i}", name=f"k{i}") for i in range(3)]
        Vp = [planes.tile([128, HW], edt, tag=f"v{i}", name=f"v{i}") for i in range(3)]
        Q = planes.tile([128, HW], f32, tag="q")
        OUT = planes.tile([128, HW], i8, tag="o")
        # int8 quantization state: per-chunk per-partition scale = 126/absmax
        am_s = singles.tile([128, 4], f32, tag="am")
        rc_s = singles.tile([128, 4], f32, tag="rc")
        nc.vector.memset(rc_s[:], 1.0)

        # ---- projections: plane m -> k/v (dual-copy weights give the same
        # output plane on partitions 0:64 and 64:128), q for m in {1,2}.
        # column-chunk OUTER so all planes' first 1536 columns (what kv chunk 0
        # needs) are projected before any plane's later columns.
        Xsrc = [t[:] for t in XP]
        with tc.tile_pool(name="pp", bufs=2, space="PSUM") as ppool:
            for base, L3 in PROJ:
                for m in range(4):
                    X = Xsrc[m]
                    projs = [(wk_s, "k"), (wv_s, "v")]
                    if m in (1, 2):
                        projs.append((wq_s, "q"))
                    for w_s, kind in projs:
                        pp = ppool.tile([128, 1536], f32, tag="pp")
                        for a, bl in _subs(L3):
                            nc.tensor.matmul(pp[:, a:a + bl], w_s[:],
                                             X[:, base + a:base + a + bl],
                                             start=True, stop=True)
                        sl = (slice(0, 64), slice(base, base + L3))
                        sh = (slice(64, 128), slice(base, base + L3))
                        if kind == "k":
                            # split k evacuations across DVE/ACT to keep DVE,
                            # the span-limiting engine, under ACT's load
                            if m <= 2:
                                nc.vector.tensor_copy(Kp[m][sl], pp[0:64, :L3])
                            if m >= 1:
                                nc.scalar.copy(Kp[m - 1][sh], pp[64:128, :L3])
                        elif kind == "v":
                            if m <= 2:
                                nc.scalar.copy(Vp[m][sl], pp[0:64, :L3])
                            if m >= 1:
                                nc.scalar.copy(Vp[m - 1][sh], pp[64:128, :L3])
                        elif m == 1:
                            nc.vector.tensor_copy(Q[sl], pp[0:64, :L3])
                        else:
                            nc.scalar.copy(Q[sh], pp[64:128, :L3])

        # ---- 27-neighbor softmax attention, PSUM-chunked over the plane
        accp = ctx.enter_context(tc.tile_pool(name="acc", bufs=1, space="PSUM"))
        OUTv = OUT.rearrange("p (r c) -> p r c", c=HP)
        GPSET = frozenset((0, 2, 6, 8, 9, 11, 15, 17, 18, 20, 21, 23, 24, 26))
        for ci, ((c0, L), (r0, r1)) in enumerate(zip(CHUNKS, OUT_ROWS)):
            den = accp.tile([128, 1536], f32, tag="den")
            num = accp.tile([128, 1536], f32, tag="num")
            for kv in range(27):
                kd, r = divmod(kv, 9)
                kh, kw = divmod(r, 3)
                dd = (kh - 1) * HP + (kw - 1)
                s_t = wpool.tile([128, 1536], f32, tag="s")
                nc.vector.scalar_tensor_tensor(
                    s_t[:, :L], Kp[kd][:, c0 + dd:c0 + dd + L],
                    b_s[:, kv:kv + 1], Q[:, c0:c0 + L], Alu.add, Alu.mult)
                e_t = wpool.tile([128, 1536], edt, tag="e")
                # bias keeps exp inside the ACT table range (softmax is
                # shift-invariant; the -28 cancels via the ln/exp normalize)
                nc.scalar.activation(e_t[:, :L], s_t[:, :L], Act.Exp, bias=ebias[:])
                ev_t = wpool.tile([128, 1536], edt, tag="ev")
                # DVE is the bottleneck engine; hand ~half the e*v products
                # to the otherwise-idle GPSIMD (stock Q7 tensor_tensor).
                ev_eng = nc.gpsimd if (kw == 1 or kv in GPSET) else nc.vector
                ev_eng.tensor_mul(ev_t[:, :L], e_t[:, :L],
                                  Vp[kd][:, c0 + dd:c0 + dd + L])
                st, sp = kv == 0, kv == 26
                for a, bl in _subs(L):
                    nc.tensor.matmul(den[:, a:a + bl], id_s[:], e_t[:, a:a + bl],
                                     start=st, stop=sp)
                    nc.tensor.matmul(num[:, a:a + bl], id_s[:], ev_t[:, a:a + bl],
                                     start=st, stop=sp)
            l_t = wpool.tile([128, 1536], f32, tag="s")
            nc.scalar.activation(l_t[:, :L], den[:, :L], Act.Ln)
            f_t = wpool.tile([128, 1536], f32, tag="f")
            nc.scalar.activation(f_t[:, :L], l_t[:, :L], Act.Exp, scale=-1.0)
            T = wpool.tile([128, 1536], f32, tag="t")
            nc.vector.tensor_mul(T[:, :L], num[:, :L], f_t[:, :L])
            # int8 quantize against this chunk's per-partition absmax; the
            # host divides by the exact same scale, so recip accuracy and the
            # 126 (vs 127) headroom only affect range, not correctness
            nc.vector.tensor_reduce(am_s[:, ci:ci + 1], T[:, :L],
                                    axis=mybir.AxisListType.X,
                                    op=Alu.max, apply_absolute_value=True)
            nc.vector.reciprocal(rc_s[:, ci:ci + 1], am_s[:, ci:ci + 1])
            nc.vector.tensor_scalar_mul(rc_s[:, ci:ci + 1],
                                        rc_s[:, ci:ci + 1], 126.0)
            nc.vector.tensor_scalar_mul(OUT[:, c0:c0 + L], T[:, :L],
                                        rc_s[:, ci:ci + 1])
            # rows fully covered by chunks <= this one stream out immediately
            nc.sync.dma_start(oc_d[:, r0 * 64:r1 * 64],
                              OUTv[:, 1 + r0:1 + r1, 1:65])
        nc.sync.dma_start(oc_d[:, 4096:4112], rc_s[:].bitcast(i8))
        # gather every core's packed result so one host fetch gets them all
        nc.gpsimd.collective_compute(
            "AllGather", mybir.AluOpType.bypass,
            replica_groups=[list(range(N_CORES))],
            ins=[oc_d[:]], outs=[og_d[:]])
        nc.sync.dma_start(out_d[:], og_d[:])
    nc.finalize()
    return nc


def _compile():
    """Build the Bass module once and cache a persistent jitted dispatcher.

    run_bass_kernel_spmd re-creates (and re-traces) its jit closure on every
    call; building it once here removes that per-call overhead and lets us
    keep the donated output buffers device-resident between calls.
    """
    import jax
    from concourse import mybir
    from concourse.bass2jax import (_bass_exec_p, partition_id_tensor,
                                    install_neuronx_cc_hook)
    from jax.sharding import Mesh, PartitionSpec, NamedSharding
    from jax.experimental.shard_map import shard_map

    install_neuronx_cc_hook()
    nc = _build()

    partition_name = nc.partition_id_tensor.name if nc.partition_id_tensor else None
    in_names, out_names, out_avals, zero_outs = [], [], [], []
    for alloc in nc.m.functions[0].allocations:
        if not isinstance(alloc, mybir.MemoryLocationSet):
            continue
        name = alloc.memorylocations[0].name
        if alloc.kind == "ExternalInput":
            if name != partition_name:
                in_names.append(name)
        elif alloc.kind == "ExternalOutput":
            shape = tuple(alloc.tensor_shape)
            dtype = mybir.dt.np(alloc.dtype)
            out_avals.append(jax.core.ShapedArray(shape, dtype))
            out_names.append(name)
            zero_outs.append(np.zeros((N_CORES * shape[0], *shape[1:]), dtype))
    n_params = len(in_names)
    n_outs = len(out_avals)
    in_names_full = list(in_names) + out_names
    if partition_name is not None:
        in_names_full.append(partition_name)
    donate = tuple(range(n_params, n_params + n_outs))

    def _body(*args):
        operands = list(args)
        if partition_name is not None:
            operands.append(partition_id_tensor())
        outs = _bass_exec_p.bind(
            *operands,
            out_avals=tuple(out_avals),
            in_names=tuple(in_names_full),
            out_names=tuple(out_names),
            lowering_input_output_aliases=(),
            sim_require_finite=True,
            sim_require_nnan=True,
            nc=nc,
        )
        return tuple(outs)

    devices = jax.devices()[:N_CORES]
    mesh = Mesh(np.asarray(devices), ("core",))
    in_specs = (PartitionSpec("core"),) * (n_params + n_outs)
    out_specs = (PartitionSpec("core"),) * n_outs
    fn = jax.jit(
        shard_map(_body, mesh=mesh, in_specs=in_specs, out_specs=out_specs,
                  check_rep=False),
        donate_argnums=donate,
        keep_unused=True,
    )
    shard = NamedSharding(mesh, PartitionSpec("core"))
    # device-resident from the start so every call (including the first) hits
    # the same jit specialization (np donation args would retrace)
    dev_zeros = [jax.device_put(z, shard) for z in zero_outs]
    _CACHE.update(nc=nc, fn=fn, in_names=in_names, prev_outs=dev_zeros,
                  n_outs=n_outs, shard=shard)


def _aux_inputs(w_q, w_k, w_v, rel_d, rel_h, rel_w):
    """Weight-dependent per-core inputs (concatenated along axis 0)."""
    rd = np.asarray(rel_d, np.float32).reshape(21, 3)
    rh = np.asarray(rel_h, np.float32).reshape(21, 3)
    rw = np.asarray(rel_w, np.float32).reshape(22, 3)

    # one-hot halo selectors over the 16 gathered planes (gathered plane j =
    # padded depth plane j+1); left halo of core i = plane 2i -> j = 2i-1,
    # right halo = plane 2i+3 -> j = 2i+2; edge cores get all-zero rows.
    hs_g = np.zeros((N_CORES, 128, 16), np.float32)
    for i in range(N_CORES):
        if i > 0:
            hs_g[i, 0:64, 2 * i - 1] = 1.0
        if i < N_CORES - 1:
            hs_g[i, 64:128, 2 * i + 2] = 1.0

    kv27 = np.arange(27)
    kd_i, kh_i, kw_i = kv27 // 9, (kv27 // 3) % 3, kv27 % 3
    B64 = np.empty((64, 27), np.float32)
    B64[:21] = rd[:, kd_i]
    B64[21:42] = rh[:, kh_i]
    B64[42:] = rw[:, kw_i]
    B = np.concatenate([B64, B64], 0)

    import ml_dtypes
    idt = np.eye(128, dtype=np.float32)
    idt = idt.astype(ml_dtypes.bfloat16 if E_BF16 else np.float32)

    def dup(w):
        w2 = np.concatenate([w.T, w.T], 1).astype(np.float16)
        return np.tile(w2, (N_CORES, 1))

    return {
        "hsel": hs_g.reshape(N_CORES * 128, 16),
        "wk2": dup(np.asarray(w_k)),
        "wv2": dup(np.asarray(w_v)),
        "wq2": dup(np.asarray(w_q)),
        "bias": np.tile(B, (N_CORES, 1)),
        "ident": np.tile(idt, (N_CORES, 1)),
    }


def _pack12(x0):
    """Quantize x to per-(depth, channel)-scaled 12-bit and pack 2 values
    into 3 bytes; the 4 trailing bytes of each row carry the f32 absmax."""
    out = np.empty((16, 64, 6148), np.uint8)

    def work(d0, d1):
        xsl = np.ascontiguousarray(
            x0[:, d0:d1].transpose(1, 0, 2, 3)).reshape(d1 - d0, 64, 4096)
        s = np.maximum(np.abs(xsl).max(axis=2), 1e-20).astype(np.float32)
        t = xsl * (np.float32(2047.0) / s)[:, :, None]
        q = (t + np.float32(2048.5)).astype(np.int16)  # floor -> round+2048
        v0 = q[:, :, 0::2]
        v1 = q[:, :, 1::2]
        pb = out[d0:d1, :, :6144].reshape(d1 - d0, 64, 2048, 3)
        pb[..., 0] = v0.astype(np.uint8)
        pb[..., 1] = ((v0 >> 8) | ((v1 & 15) << 4)).astype(np.uint8)
        pb[..., 2] = (v1 >> 4).astype(np.uint8)
        out[d0:d1, :, 6144:] = s.view(np.uint8).reshape(d1 - d0, 64, 4)

    futs = [_POOL.submit(work, d, d + 2) for d in range(0, 16, 2)]
    for f in futs:
        f.result()
    return out.reshape(16 * 64, 6148)


def kernel(x, w_q, w_k, w_v, rel_d, rel_h, rel_w):
    import jax
    import hashlib

    if "fn" not in _CACHE:
        _compile()

    x = np.asarray(x, np.float32)
    # core i's own planes, partition = slot*64 + channel, packed 12-bit
    bnd = _pack12(np.asarray(x[0]))

    # weights/bias/ident/hsel are tiny but cost per-shard transfer overhead;
    # keep them device-resident across calls, re-uploading only if changed
    h = hashlib.sha1()
    for a in (w_q, w_k, w_v, rel_d, rel_h, rel_w):
        h.update(np.ascontiguousarray(a).tobytes())
    key = h.hexdigest()
    if _CACHE.get("aux_key") != key:
        aux = _aux_inputs(w_q, w_k, w_v, rel_d, rel_h, rel_w)
        _CACHE["aux_dev"] = {k: jax.device_put(v, _CACHE["shard"])
                             for k, v in aux.items()}
        _CACHE["aux_key"] = key
    gmaps = dict(_CACHE["aux_dev"])
    gmaps["bnd"] = bnd

    args = [gmaps[nm] for nm in _CACHE["in_names"]]
    out_arrs = _CACHE["fn"](*args, *_CACHE["prev_outs"])
    # recycle the device-resident output buffers as next call's donation args
    # (their contents are irrelevant: the NEFF writes every output element)
    _CACHE["prev_outs"] = list(out_arrs)

    # every core holds the full gathered result; fetch only device 0's shard
    o = np.asarray(out_arrs[0].addressable_shards[0].data)  # [8*128, 4112] i8
    sc = np.ascontiguousarray(o[:, 4096:4112]).view(np.float32)  # [8*128, 4]
    o8 = o[:, :4096].reshape(N_CORES * 128, 64, 64)
    band = np.empty(64, np.int64)
    for ci, (r0, r1) in enumerate(OUT_ROWS):
        band[r0:r1] = ci
    rsc = np.float32(1.0) / sc[:, band]
    of = np.multiply(o8, rsc[:, :, None], dtype=np.float32)
    out = np.empty((1, 64, 16, 64, 64), np.float32)
    out.reshape(64, N_CORES, 2, 64, 64)[:] = \
        of.reshape(N_CORES, 2, 64, 64, 64).transpose(2, 0, 1, 3, 4)
    return out


# revision 36
# speedup vs baseline: 1.9518x; 1.1142x over previous
"""AttentionConv3D Trainium2 kernel.

Computation (per channel c, voxel (d,h,w)):
    q,k,v = 1x1x1 convs of x;  s_kv = q * (k_pad[nbr kv] + rel_bias(c,kv))
    out   = sum_kv softmax_kv(s) * v_pad[nbr kv]         (27 = 3x3x3 window)

Strategy: depth-shard over 8 cores (2 output depth planes each, 1-plane halo).
On-device layout: partition p = dl*64 + c (dl in {0,1} local depth), free dim
= zero-padded 66x66 plane (4356). Per kv-neighbor the window access is a
free-dim offset (kh-1)*66 + (kw-1) into one of three depth-plane buffers
K[kd]; the rel bias collapses to a per-partition scalar B[p, kv], so
s = (K_shift + B)*q is ONE DVE scalar_tensor_tensor op. exp on ACT;
numerator/denominator accumulated with identity matmuls into PSUM on the
TensorEngine; 1/den via exp(-ln(den)) on ACT.

The wall clock is dominated by the ~40MB/s (half-duplex) axon tunnel, so I/O
is minimized:
 - each core uploads ONLY its two unpadded fp16 depth planes (8.4MB total,
   no halo duplication, padding assembled on device); the 1-plane halos are
   exchanged on device: a world AllGather of every core's plane pair, then a
   per-core one-hot masked sum (host-uploaded selection scalars, 16 DVE
   select-accumulate ops) picks the two neighbor planes — edge cores get
   all-zero masks, i.e. free zero padding.
 - projection matmuls run fp16 x fp16 -> fp32 PSUM.
 - the output is quantized on device to int8 against a per-(partition, row
   band) absmax scale (the host divides by the exact same downloaded scale),
   AllGathered across cores, and fetched as ONE 4.3MB transfer from core 0's
   shard instead of 16 per-shard fetches.
 - weights/bias/identity/halo-selectors are kept device-resident across
   calls (sha1 of the weight args guards staleness).
 - donated output buffers are device-resident (recycled between calls).
 - the jitted dispatch closure is built once and cached.
"""

import sys
import numpy as np

for _p in ("/opt/trn_rl_repo", "/root/.axon_site/_ro/trn_rl_repo"):
    if _p not in sys.path:
        sys.path.insert(0, _p)

HP = 66               # padded plane edge
HW = HP * HP          # 4356
CHUNKS = [(67, 1386), (1453, 1452), (2905, 1384)]  # covers [67, 4289), chunk
# edges row-aligned so each output row band is quantized by exactly one
# chunk's int8 scale. Chunk 0's window reads ([0, 1520)) still fit inside proj
# col-chunk 0 so the kv loop overlaps the tail of the projection phase.
PROJ = [(0, 1536), (1536, 1536), (3072, 1284)]     # proj psum chunks over 4356
OUT_ROWS = [(0, 21), (21, 43), (43, 64)]           # row bands DMA'd per chunk
ZW = 512              # halo-assembly column chunk (8 chunks cover 4096)

# hot-path dtype knobs (fp32 = safe; bf16 halves DVE cost of the e*v path)
E_BF16 = True   # e / v / ev tiles + identity in bf16 (PE still accums fp32)

N_CORES = 8

_CACHE = {}

from concurrent.futures import ThreadPoolExecutor
_POOL = ThreadPoolExecutor(8)


def _subs(L):
    return [(0, 512), (512, 512), (1024, L - 1024)]


def _build():
    from contextlib import ExitStack
    import concourse.bacc as bacc
    import concourse.tile as tile
    from concourse import mybir

    f32 = mybir.dt.float32
    f16 = mybir.dt.float16
    bf16 = mybir.dt.bfloat16
    edt = bf16 if E_BF16 else f32
    Alu = mybir.AluOpType
    Act = mybir.ActivationFunctionType

    nc = bacc.Bacc("TRN2", target_bir_lowering=False)
    u8 = mybir.dt.uint8
    # own two unpadded depth planes, partition = slot*64 + channel, packed as
    # 12-bit fixed point (2 values per 3 bytes) + 4 trailing bytes holding the
    # per-partition f32 dequant range (absmax)
    bnd_d = nc.dram_tensor("bnd", [128, 6148], u8, kind="ExternalInput")
    # halo selection scalars: rows 0:64 pick the left-halo plane, 64:128 the
    # right-halo plane, as one-hot over the 16 gathered planes
    hs_d = nc.dram_tensor("hsel", [128, 16], f32, kind="ExternalInput")
    wk_d = nc.dram_tensor("wk2", [64, 128], f16, kind="ExternalInput")
    wv_d = nc.dram_tensor("wv2", [64, 128], f16, kind="ExternalInput")
    wq_d = nc.dram_tensor("wq2", [64, 128], f16, kind="ExternalInput")
    b_d = nc.dram_tensor("bias", [128, 27], f32, kind="ExternalInput")
    id_d = nc.dram_tensor("ident", [128, 128], edt, kind="ExternalInput")
    i8 = mybir.dt.int8
    # every core receives the full gathered result; the host fetches only
    # core 0's copy (one transfer instead of 16 per-shard fetches)
    out_d = nc.dram_tensor("out", [N_CORES * 128, 4112], i8,
                           kind="ExternalOutput")

    # collective staging (collectives can't touch I/O tensors directly)
    bb_d = nc.dram_tensor("bb", [128, 4096], f16)
    g_d = nc.dram_tensor("g", [16, 64, 4096], f16, addr_space="Shared")
    # per-core packed result: 64*64 int8 voxels + 16 bytes of f32 scales
    oc_d = nc.dram_tensor("oc", [128, 4112], i8)
    og_d = nc.dram_tensor("og", [N_CORES * 128, 4112], i8, addr_space="Shared")

    with tile.TileContext(nc) as tc, ExitStack() as ctx:
        singles = ctx.enter_context(tc.tile_pool(name="singles", bufs=1))
        planes = ctx.enter_context(tc.tile_pool(name="planes", bufs=1))
        wpool = ctx.enter_context(tc.tile_pool(name="work", bufs=2))

        wk_s = singles.tile([64, 128], f16, tag="wk")
        wv_s = singles.tile([64, 128], f16, tag="wv")
        wq_s = singles.tile([64, 128], f16, tag="wq")
        id_s = singles.tile([128, 128], edt, tag="id")
        b_s = singles.tile([128, 27], f32, tag="b")
        hs_s = singles.tile([128, 16], f32, tag="hs")
        ebias = singles.tile([128, 1], f32, tag="ebias")
        nc.vector.memset(ebias[:], -28.0)
        for t, d in ((wk_s, wk_d), (wv_s, wv_d), (wq_s, wq_d),
                     (id_s, id_d), (b_s, b_d), (hs_s, hs_d)):
            nc.sync.dma_start(t[:], d[:])

        # the four padded depth planes, assembled from unpadded data: memset
        # zeroes the 66x66 borders, the interiors land via strided DMAs
        XP = [planes.tile([64, HW], f16, tag=f"xp{m}", name=f"xp{m}")
              for m in range(4)]
        XPv = [t.rearrange("p (r c) -> p r c", c=HP) for t in XP]
        for t in XP:
            nc.vector.memset(t[:], 0.0)

        # ---- unpack the 12-bit planes to fp16 (float-domain bit math is
        # exact for 12-bit ints), then halo-exchange the fp16 result
        with tc.tile_pool(name="up", bufs=1) as upool:
            XU = upool.tile([128, 4096], f16, tag="xu")
            XUv = XU.rearrange("p (n t) -> p n t", t=2)
            U8s = upool.tile([128, 4], u8, tag="u8s")
            nc.sync.dma_start(U8s[:], bnd_d[:, 6144:6148])
            m_s = singles.tile([128, 1], f32, tag="m")
            mb_s = singles.tile([128, 1], f32, tag="mb")
            nc.vector.tensor_scalar_mul(m_s[:], U8s[:].bitcast(f32),
                                        1.0 / 2047.0)
            nc.vector.tensor_scalar_mul(mb_s[:], m_s[:], -2048.0)
            for c in range(4):
                U8 = upool.tile([128, 1536], u8, tag="u8")
                nc.sync.dma_start(U8[:], bnd_d[:, c * 1536:(c + 1) * 1536])
                U3 = U8[:].rearrange("p (n t) -> p n t", t=3)
                b0f = upool.tile([128, 512], f32, tag="b0")
                b1f = upool.tile([128, 512], f32, tag="b1")
                b2f = upool.tile([128, 512], f32, tag="b2")
                nc.scalar.copy(b0f[:], U3[:, :, 0])
                nc.scalar.copy(b1f[:], U3[:, :, 1])
                nc.scalar.copy(b2f[:], U3[:, :, 2])
                # hi = b1 >> 4 without mod (invalid on TensorScalar): the
                # offset -15/32 makes round-to-nearest(b1/16 - 0.46875) hit
                # floor(b1/16) exactly for every 4-bit fraction (never a tie)
                t_f = upool.tile([128, 512], f32, tag="t")
                nc.vector.tensor_scalar(t_f[:], b1f[:], 0.0625, -0.46875,
                                        Alu.mult, Alu.add)
                hi_i = upool.tile([128, 512], mybir.dt.int16, tag="hi")
                nc.vector.tensor_copy(hi_i[:], t_f[:])   # f32->i16 rounds
                hi_f = upool.tile([128, 512], f32, tag="hf")
                nc.vector.tensor_copy(hi_f[:], hi_i[:])
                # v0 = (b0 + 256*b1) - 4096*hi;  v1 = hi + 16*b2
                v0 = upool.tile([128, 512], f32, tag="v0")
                v1 = upool.tile([128, 512], f32, tag="v1")
                nc.vector.scalar_tensor_tensor(v0[:], b1f[:], 256.0, b0f[:],
                                               Alu.mult, Alu.add)
                nc.vector.scalar_tensor_tensor(v0[:], hi_f[:], -4096.0, v0[:],
                                               Alu.mult, Alu.add)
                nc.vector.scalar_tensor_tensor(v1[:], b2f[:], 16.0, hi_f[:],
                                               Alu.mult, Alu.add)
                sl = slice(c * 512, (c + 1) * 512)
                nc.vector.tensor_scalar(XUv[:, sl, 0], v0[:], m_s[:], mb_s[:],
                                        Alu.mult, Alu.add)
                nc.vector.tensor_scalar(XUv[:, sl, 1], v1[:], m_s[:], mb_s[:],
                                        Alu.mult, Alu.add)
            # own-plane interiors + the collective bounce, straight from XU
            XUr = XU.rearrange("p (r c) -> p r c", c=64)
            nc.sync.dma_start(XPv[1][:, 1:65, 1:65], XUr[0:64])
            nc.sync.dma_start(XPv[2][:, 1:65, 1:65], XUr[64:128])
            nc.gpsimd.dma_start(bb_d[:], XU[:])

        # ---- halo exchange: world AllGather of everyone's plane pair
        nc.gpsimd.collective_compute(
            "AllGather", mybir.AluOpType.bypass,
            replica_groups=[list(range(N_CORES))],
            ins=[bb_d[:]], outs=[g_d[:]])

        with tc.tile_pool(name="gt", bufs=1) as gpool:
            # 0:64 = left halo plane, 64:128 = right halo plane (unpadded)
            XH = gpool.tile([128, 4096], f16, tag="xh")
            for w in range(8):
                ws = w * ZW
                GT = gpool.tile([128, 16 * ZW], f16, tag="gt")
                GTv = GT.rearrange("p (j z) -> p j z", j=16)
                src = g_d[:, :, ws:ws + ZW].transpose([1, 0, 2])
                nc.sync.dma_start(GTv[0:64, :, :], src)
                nc.sync.dma_start(GTv[64:128, :, :], src)
                # one-hot select-accumulate over the 16 gathered planes
                nc.vector.tensor_scalar_mul(
                    XH[:, ws:ws + ZW], GT[:, 0:ZW], hs_s[:, 0:1])
                for j in range(1, 16):
                    nc.vector.scalar_tensor_tensor(
                        XH[:, ws:ws + ZW], GT[:, j * ZW:(j + 1) * ZW],
                        hs_s[:, j:j + 1], XH[:, ws:ws + ZW],
                        Alu.mult, Alu.add)
            # pad the halo planes (and bring the right halo down to base
            # partition 0, which matmul moving operands require)
            XHv = XH.rearrange("p (r c) -> p r c", c=64)
            nc.sync.dma_start(XPv[0][:, 1:65, 1:65], XHv[0:64])
            nc.sync.dma_start(XPv[3][:, 1:65, 1:65], XHv[64:128])

        Kp = [planes.tile([128, HW], f32, tag=f"k{i}", name=f"k{i}") for i in range(3)]
        Vp = [planes.tile([128, HW], edt, tag=f"v{i}", name=f"v{i}") for i in range(3)]
        Q = planes.tile([128, HW], f32, tag="q")
        OUT = planes.tile([128, HW], i8, tag="o")
        # int8 quantization state: per-chunk per-partition scale = 126/absmax
        am_s = singles.tile([128, 4], f32, tag="am")
        rc_s = singles.tile([128, 4], f32, tag="rc")
        nc.vector.memset(rc_s[:], 1.0)

        # ---- projections: plane m -> k/v (dual-copy weights give the same
        # output plane on partitions 0:64 and 64:128), q for m in {1,2}.
        # column-chunk OUTER so all planes' first 1536 columns (what kv chunk 0
        # needs) are projected before any plane's later columns.
        Xsrc = [t[:] for t in XP]
        with tc.tile_pool(name="pp", bufs=2, space="PSUM") as ppool:
            for base, L3 in PROJ:
                for m in range(4):
                    X = Xsrc[m]
                    projs = [(wk_s, "k"), (wv_s, "v")]
                    if m in (1, 2):
                        projs.append((wq_s, "q"))
                    for w_s, kind in projs:
                        pp = ppool.tile([128, 1536], f32, tag="pp")
                        for a, bl in _subs(L3):
                            nc.tensor.matmul(pp[:, a:a + bl], w_s[:],
                                             X[:, base + a:base + a + bl],
                                             start=True, stop=True)
                        sl = (slice(0, 64), slice(base, base + L3))
                        sh = (slice(64, 128), slice(base, base + L3))
                        if kind == "k":
                            # split k evacuations across DVE/ACT to keep DVE,
                            # the span-limiting engine, under ACT's load
                            if m <= 2:
                                nc.vector.tensor_copy(Kp[m][sl], pp[0:64, :L3])
                            if m >= 1:
                                nc.scalar.copy(Kp[m - 1][sh], pp[64:128, :L3])
                        elif kind == "v":
                            if m <= 2:
                                nc.scalar.copy(Vp[m][sl], pp[0:64, :L3])
                            if m >= 1:
                                nc.scalar.copy(Vp[m - 1][sh], pp[64:128, :L3])
                        elif m == 1:
                            nc.vector.tensor_copy(Q[sl], pp[0:64, :L3])
                        else:
                            nc.scalar.copy(Q[sh], pp[64:128, :L3])

        # ---- 27-neighbor softmax attention, PSUM-chunked over the plane
        accp = ctx.enter_context(tc.tile_pool(name="acc", bufs=1, space="PSUM"))
        OUTv = OUT.rearrange("p (r c) -> p r c", c=HP)
        GPSET = frozenset((0, 2, 6, 8, 9, 11, 15, 17, 18, 20, 21, 23, 24, 26))
        for ci, ((c0, L), (r0, r1)) in enumerate(zip(CHUNKS, OUT_ROWS)):
            den = accp.tile([128, 1536], f32, tag="den")
            num = accp.tile([128, 1536], f32, tag="num")
            for kv in range(27):
                kd, r = divmod(kv, 9)
                kh, kw = divmod(r, 3)
                dd = (kh - 1) * HP + (kw - 1)
                s_t = wpool.tile([128, 1536], f32, tag="s")
                nc.vector.scalar_tensor_tensor(
                    s_t[:, :L], Kp[kd][:, c0 + dd:c0 + dd + L],
                    b_s[:, kv:kv + 1], Q[:, c0:c0 + L], Alu.add, Alu.mult)
                e_t = wpool.tile([128, 1536], edt, tag="e")
                # bias keeps exp inside the ACT table range (softmax is
                # shift-invariant; the -28 cancels via the ln/exp normalize)
                nc.scalar.activation(e_t[:, :L], s_t[:, :L], Act.Exp, bias=ebias[:])
                ev_t = wpool.tile([128, 1536], edt, tag="ev")
                # DVE is the bottleneck engine; hand ~half the e*v products
                # to the otherwise-idle GPSIMD (stock Q7 tensor_tensor).
                ev_eng = nc.gpsimd if (kw == 1 or kv in GPSET) else nc.vector
                ev_eng.tensor_mul(ev_t[:, :L], e_t[:, :L],
                                  Vp[kd][:, c0 + dd:c0 + dd + L])
                st, sp = kv == 0, kv == 26
                for a, bl in _subs(L):
                    nc.tensor.matmul(den[:, a:a + bl], id_s[:], e_t[:, a:a + bl],
                                     start=st, stop=sp)
                    nc.tensor.matmul(num[:, a:a + bl], id_s[:], ev_t[:, a:a + bl],
                                     start=st, stop=sp)
            l_t = wpool.tile([128, 1536], f32, tag="s")
            nc.scalar.activation(l_t[:, :L], den[:, :L], Act.Ln)
            f_t = wpool.tile([128, 1536], f32, tag="f")
            nc.scalar.activation(f_t[:, :L], l_t[:, :L], Act.Exp, scale=-1.0)
            T = wpool.tile([128, 1536], f32, tag="t")
            nc.vector.tensor_mul(T[:, :L], num[:, :L], f_t[:, :L])
            # int8 quantize against this chunk's per-partition absmax; the
            # host divides by the exact same scale, so recip accuracy and the
            # 126 (vs 127) headroom only affect range, not correctness
            nc.vector.tensor_reduce(am_s[:, ci:ci + 1], T[:, :L],
                                    axis=mybir.AxisListType.X,
                                    op=Alu.max, apply_absolute_value=True)
            nc.vector.reciprocal(rc_s[:, ci:ci + 1], am_s[:, ci:ci + 1])
            nc.vector.tensor_scalar_mul(rc_s[:, ci:ci + 1],
                                        rc_s[:, ci:ci + 1], 126.0)
            nc.vector.tensor_scalar_mul(OUT[:, c0:c0 + L], T[:, :L],
                                        rc_s[:, ci:ci + 1])
            # rows fully covered by chunks <= this one stream out immediately
            nc.sync.dma_start(oc_d[:, r0 * 64:r1 * 64],
                              OUTv[:, 1 + r0:1 + r1, 1:65])
        nc.sync.dma_start(oc_d[:, 4096:4112], rc_s[:].bitcast(i8))
        # gather every core's packed result so one host fetch gets them all
        nc.gpsimd.collective_compute(
            "AllGather", mybir.AluOpType.bypass,
            replica_groups=[list(range(N_CORES))],
            ins=[oc_d[:]], outs=[og_d[:]])
        nc.sync.dma_start(out_d[:], og_d[:])
    nc.finalize()
    return nc


def _compile():
    """Build the Bass module once and cache a persistent jitted dispatcher.

    run_bass_kernel_spmd re-creates (and re-traces) its jit closure on every
    call; building it once here removes that per-call overhead and lets us
    keep the donated output buffers device-resident between calls.
    """
    import jax
    from concourse import mybir
    from concourse.bass2jax import (_bass_exec_p, partition_id_tensor,
                                    install_neuronx_cc_hook)
    from jax.sharding import Mesh, PartitionSpec, NamedSharding
    from jax.experimental.shard_map import shard_map

    install_neuronx_cc_hook()
    nc = _build()

    partition_name = nc.partition_id_tensor.name if nc.partition_id_tensor else None
    in_names, out_names, out_avals, zero_outs = [], [], [], []
    for alloc in nc.m.functions[0].allocations:
        if not isinstance(alloc, mybir.MemoryLocationSet):
            continue
        name = alloc.memorylocations[0].name
        if alloc.kind == "ExternalInput":
            if name != partition_name:
                in_names.append(name)
        elif alloc.kind == "ExternalOutput":
            shape = tuple(alloc.tensor_shape)
            dtype = mybir.dt.np(alloc.dtype)
            out_avals.append(jax.core.ShapedArray(shape, dtype))
            out_names.append(name)
            zero_outs.append(np.zeros((N_CORES * shape[0], *shape[1:]), dtype))
    n_params = len(in_names)
    n_outs = len(out_avals)
    in_names_full = list(in_names) + out_names
    if partition_name is not None:
        in_names_full.append(partition_name)
    donate = tuple(range(n_params, n_params + n_outs))

    def _body(*args):
        operands = list(args)
        if partition_name is not None:
            operands.append(partition_id_tensor())
        outs = _bass_exec_p.bind(
            *operands,
            out_avals=tuple(out_avals),
            in_names=tuple(in_names_full),
            out_names=tuple(out_names),
            lowering_input_output_aliases=(),
            sim_require_finite=True,
            sim_require_nnan=True,
            nc=nc,
        )
        return tuple(outs)

    devices = jax.devices()[:N_CORES]
    mesh = Mesh(np.asarray(devices), ("core",))
    in_specs = (PartitionSpec("core"),) * (n_params + n_outs)
    out_specs = (PartitionSpec("core"),) * n_outs
    fn = jax.jit(
        shard_map(_body, mesh=mesh, in_specs=in_specs, out_specs=out_specs,
                  check_rep=False),
        donate_argnums=donate,
        keep_unused=True,
    )
    shard = NamedSharding(mesh, PartitionSpec("core"))
    # device-resident from the start so every call (including the first) hits
    # the same jit specialization (np donation args would retrace)
    dev_zeros = [jax.device_put(z, shard) for z in zero_outs]
    _CACHE.update(nc=nc, fn=fn, in_names=in_names, prev_outs=dev_zeros,
                  n_outs=n_outs, shard=shard, devices=list(devices))


def _aux_inputs(w_q, w_k, w_v, rel_d, rel_h, rel_w):
    """Weight-dependent per-core inputs (concatenated along axis 0)."""
    rd = np.asarray(rel_d, np.float32).reshape(21, 3)
    rh = np.asarray(rel_h, np.float32).reshape(21, 3)
    rw = np.asarray(rel_w, np.float32).reshape(22, 3)

    # one-hot halo selectors over the 16 gathered planes (gathered plane j =
    # padded depth plane j+1); left halo of core i = plane 2i -> j = 2i-1,
    # right halo = plane 2i+3 -> j = 2i+2; edge cores get all-zero rows.
    hs_g = np.zeros((N_CORES, 128, 16), np.float32)
    for i in range(N_CORES):
        if i > 0:
            hs_g[i, 0:64, 2 * i - 1] = 1.0
        if i < N_CORES - 1:
            hs_g[i, 64:128, 2 * i + 2] = 1.0

    kv27 = np.arange(27)
    kd_i, kh_i, kw_i = kv27 // 9, (kv27 // 3) % 3, kv27 % 3
    B64 = np.empty((64, 27), np.float32)
    B64[:21] = rd[:, kd_i]
    B64[21:42] = rh[:, kh_i]
    B64[42:] = rw[:, kw_i]
    B = np.concatenate([B64, B64], 0)

    import ml_dtypes
    idt = np.eye(128, dtype=np.float32)
    idt = idt.astype(ml_dtypes.bfloat16 if E_BF16 else np.float32)

    def dup(w):
        w2 = np.concatenate([w.T, w.T], 1).astype(np.float16)
        return np.tile(w2, (N_CORES, 1))

    return {
        "hsel": hs_g.reshape(N_CORES * 128, 16),
        "wk2": dup(np.asarray(w_k)),
        "wv2": dup(np.asarray(w_v)),
        "wq2": dup(np.asarray(w_q)),
        "bias": np.tile(B, (N_CORES, 1)),
        "ident": np.tile(idt, (N_CORES, 1)),
    }


def _pack12_core(x0, i):
    """Pack core i's two depth planes to 12-bit (2 values per 3 bytes, 4
    trailing bytes per row = f32 absmax) and start its device upload."""
    import jax
    xsl = np.ascontiguousarray(
        x0[:, 2 * i:2 * i + 2].transpose(1, 0, 2, 3)).reshape(2, 64, 4096)
    s = np.maximum(np.abs(xsl).max(axis=2), 1e-20).astype(np.float32)
    t = xsl * (np.float32(2047.0) / s)[:, :, None]
    q = (t + np.float32(2048.5)).astype(np.int16)  # floor -> round+2048
    v0 = q[:, :, 0::2]
    v1 = q[:, :, 1::2]
    out = np.empty((2, 64, 6148), np.uint8)
    pb = out[:, :, :6144].reshape(2, 64, 2048, 3)
    pb[..., 0] = v0.astype(np.uint8)
    pb[..., 1] = ((v0 >> 8) | ((v1 & 15) << 4)).astype(np.uint8)
    pb[..., 2] = (v1 >> 4).astype(np.uint8)
    out[:, :, 6144:] = s.view(np.uint8).reshape(2, 64, 4)
    # async put: the upload streams while other cores are still packing
    return jax.device_put(out.reshape(128, 6148), _CACHE["devices"][i])


def kernel(x, w_q, w_k, w_v, rel_d, rel_h, rel_w):
    import jax
    import hashlib

    if "fn" not in _CACHE:
        _compile()

    x = np.asarray(x, np.float32)
    # core i's own planes, partition = slot*64 + channel, packed 12-bit;
    # per-core pack workers overlap packing with the (serialized) uploads
    x0 = np.asarray(x[0])
    futs = [_POOL.submit(_pack12_core, x0, i) for i in range(N_CORES)]
    bnd = jax.make_array_from_single_device_arrays(
        (N_CORES * 128, 6148), _CACHE["shard"], [f.result() for f in futs])

    # weights/bias/ident/hsel are tiny but cost per-shard transfer overhead;
    # keep them device-resident across calls, re-uploading only if changed
    h = hashlib.sha1()
    for a in (w_q, w_k, w_v, rel_d, rel_h, rel_w):
        h.update(np.ascontiguousarray(a).tobytes())
    key = h.hexdigest()
    if _CACHE.get("aux_key") != key:
        aux = _aux_inputs(w_q, w_k, w_v, rel_d, rel_h, rel_w)
        _CACHE["aux_dev"] = {k: jax.device_put(v, _CACHE["shard"])
                             for k, v in aux.items()}
        _CACHE["aux_key"] = key
    gmaps = dict(_CACHE["aux_dev"])
    gmaps["bnd"] = bnd

    args = [gmaps[nm] for nm in _CACHE["in_names"]]
    out_arrs = _CACHE["fn"](*args, *_CACHE["prev_outs"])
    # recycle the device-resident output buffers as next call's donation args
    # (their contents are irrelevant: the NEFF writes every output element)
    _CACHE["prev_outs"] = list(out_arrs)

    # every core holds the full gathered result; fetch only device 0's shard
    o = np.asarray(out_arrs[0].addressable_shards[0].data)  # [8*128, 4112] i8
    sc = np.ascontiguousarray(o[:, 4096:4112]).view(np.float32)  # [8*128, 4]
    o8 = o[:, :4096].reshape(N_CORES, 2, 64, 64, 64)  # (i, slot, c, h, w)
    band = np.empty(64, np.int64)
    for ci, (r0, r1) in enumerate(OUT_ROWS):
        band[r0:r1] = ci
    rsc = (np.float32(1.0) / sc[:, band]).reshape(N_CORES, 2, 64, 64)
    out = np.empty((1, 64, 16, 64, 64), np.float32)
    ov = out[0].reshape(64, N_CORES, 2, 64, 64)

    def dequant(i):
        # fused int8 -> f32 dequant writing straight into the output layout
        np.multiply(o8[i].transpose(1, 0, 2, 3),
                    rsc[i].transpose(1, 0, 2)[:, :, :, None],
                    out=ov[:, i], dtype=np.float32)

    for f in [_POOL.submit(dequant, i) for i in range(N_CORES)]:
        f.result()
    return out
